# revision 39
# baseline (speedup 1.0000x reference)
"""Multi-head self-attention (B=4, S=2048, D=1024, H=8) on 8 TRN2 NeuronCores.

Sharding: core c -> batch b=c//2, head-group g=c%2 (4 heads/core).
Each core computes its 4 heads' attention output [512, 2048] (transposed,
head-major); the host gathers/reassembles the full [B, S, D] output.

Notes on the math: the reference adds the source mask per-QUERY (constant
along the key axis) before a softmax over keys, so the mask cancels exactly;
encoder_output_embedding and the target mask are unused by the reference.
The kernel therefore computes pure softmax(q k^T / sqrt(dh)) v.

Perf structure (per core, PE stream is the roofline: ~896 f16 matmuls of
128x128x512 running back-to-back at ~1 column/cycle):
- input DMAs are issued in strict consumption order on the two HWDGE
  queues only (wq/wk last — a third queue steals HBM bandwidth from the
  V-pass stream, which runs at HBM saturation); the first granules are
  half-size so the V pass starts as early as the cold DMA rate allows,
  with warm-up matmuls covering the wait and ramping the PE clock;
- softmax exp-sums fold on DVE right after each EXP (decoupled from the
  PV matmuls), so the per-block sum matmuls never wait on the PV chain;
- the sum matmuls of the first 7 blocks are deferred into the LAST
  attention block, whose inner loop is ACT(exp)-paced with idle PE
  slots — there they cost nothing; their DMAs ride the sync queue so
  the scalar sequencer (which dispatches the EXPs) stays clear;
- the V projection uses 6 PSUM accumulator banks (3 passes) with the
  projection PSUM pool opened up front, so head 0's first projection
  matmul never waits on the V pool-close barrier;
- the device stores the UNNORMALIZED attention output (f16) plus the
  per-block exp-sums ([1,1024] f32 per block); the softmax division
  happens on the host during unsharding. This removes the reciprocal/
  broadcast/multiply chain from the device tail entirely — after the
  last PV matmul only a PSUM->f16 cast (split ACT/DVE) and the final
  stores (split across both HWDGE queues) remain.
"""

import math
from contextlib import ExitStack

import numpy as np

import concourse.bacc as bacc
import concourse.tile as tile
from concourse import mybir
from concourse.bass_utils import run_bass_kernel_spmd

N_CORES = 8
B, S, D, H = 4, 2048, 1024, 8
DH = 128                    # head dim
HPC = 4                     # heads per core
DHG = HPC * DH              # 512: projected width per core
SCALE = 1.0 / math.sqrt(DH)

F32 = mybir.dt.float32
F16 = mybir.dt.float16

TRACE = False               # test.py flips this for profiling runs
_CACHE = {}


def _emit(tc, nc, xt_ap, wq_ap, wk_ap, wv_ap, out_ap, sums_ap):
    KT = S // 128            # 16 key tiles
    ND = D // 128            # 8 contraction tiles

    with ExitStack() as ctx:
        p_xt = ctx.enter_context(tc.tile_pool(name="xt", bufs=ND))
        p_w = ctx.enter_context(tc.tile_pool(name="w", bufs=2))
        p_qt = ctx.enter_context(tc.tile_pool(name="qt", bufs=2))
        p_v = ctx.enter_context(tc.tile_pool(name="v", bufs=KT))
        p_exp = ctx.enter_context(tc.tile_pool(name="exp", bufs=6))
        p_out = ctx.enter_context(tc.tile_pool(name="o", bufs=2))
        p_rc = ctx.enter_context(tc.tile_pool(name="rc", bufs=2))
        p_const = ctx.enter_context(tc.tile_pool(name="const", bufs=1))

        ones = p_const.tile([128, 1], F16, tag="ones")
        nc.vector.memset(ones[:], 1.0)
        # warm-up matmuls may read mostly-uninitialized SBUF — their
        # output region is dead (overwritten by the V pass's start=True
        # matmuls) — so only a 1-column memset allocates the tile and the
        # first warm-up issues ~0.4us sooner.
        scratch = p_const.tile([128, 512], F16, tag="scratch")
        nc.vector.memset(scratch[:, 0:1], 0.0)

        # DMA cost here is DESCRIPTOR-processing-bound (~155ns/desc, one
        # desc per partition row), so inputs use host-prearranged "panel"
        # layouts giving 4KB contiguous per partition line. Issue order
        # matches consumption order: the V pass (first PE work) consumes
        # (wv[d], xt[d]) for d ascending on the two HWDGE queues; the
        # later-needed wk goes on the SWDGE (gpsimd) queue.
        xts = [[None] * (S // 512) for _ in range(ND)]
        ws = {"wv": [None] * ND, "wq": [None] * ND, "wk": [None] * ND}

        def dma_w(eng, name, ap, c0, nch):
            t = p_w.tile(
                [128, nch * DHG], F16, tag=name, name=f"{name}c{c0}",
                bufs=6 if name == "wv" else 2,
            )
            eng.dma_start(t[:], ap[:, c0 * DHG:(c0 + nch) * DHG])
            for j in range(nch):
                ws[name][c0 + j] = t[:, j * DHG:(j + 1) * DHG]

        def dma_xt(eng, d, half=None):
            # full 4KB row per partition (half the descriptors of a split
            # load, so the HWDGE queues sustain the V-pass feed rate);
            # the first two d-chunks load as h0/h1 halves so pass A's
            # first accumulation groups start ~1.5us earlier.
            if half is None:
                t = p_xt.tile([128, S], F16, tag="xt", name=f"xtd{d}")
                eng.dma_start(t[:], xt_ap[:, d * S:(d + 1) * S])
                for sb in range(S // 512):
                    xts[d][sb] = t[:, sb * 512:(sb + 1) * 512]
            else:
                t = p_xt.tile(
                    [128, S // 2], F16, tag="xth", name=f"xtd{d}h{half}", bufs=4
                )
                eng.dma_start(
                    t[:],
                    xt_ap[:, d * S + half * 1024:d * S + (half + 1) * 1024],
                )
                xts[d][2 * half] = t[:, 0:512]
                xts[d][2 * half + 1] = t[:, 512:1024]

        # Strict first-need order, HWDGE queues only (a third queue would
        # steal HBM bandwidth from the critical V-pass stream — the input
        # load runs at HBM saturation for ~17us).
        dma_w(nc.sync, "wv", wv_ap, 0, 2)
        dma_xt(nc.scalar, 0, 0)
        dma_xt(nc.sync, 1, 0)
        dma_w(nc.scalar, "wv", wv_ap, 2, 2)
        dma_xt(nc.sync, 2)
        dma_xt(nc.scalar, 3)
        dma_w(nc.sync, "wv", wv_ap, 4, 2)
        dma_xt(nc.scalar, 4)
        dma_w(nc.scalar, "wv", wv_ap, 6, 2)
        dma_xt(nc.sync, 5)
        dma_xt(nc.scalar, 6)
        dma_xt(nc.sync, 7)
        dma_xt(nc.scalar, 0, 1)
        dma_xt(nc.sync, 1, 1)
        dma_w(nc.scalar, "wq", wq_ap, 0, 4)
        dma_w(nc.sync, "wq", wq_ap, 4, 4)
        dma_w(nc.scalar, "wk", wk_ap, 0, 4)
        dma_w(nc.sync, "wk", wk_ap, 4, 4)

        # V = x @ wv in two d-outer passes over 8 PSUM accumulators: the
        # first accumulation group needs only (wv half 0, xt chunk 0) so
        # the PE starts right after the fixed preamble. The pool closes
        # before the attention pools claim the banks.
        # ps_pj opens BEFORE the V pool so its banks are disjoint from
        # V's accumulators: head 0's first projection matmul then never
        # waits on the V pool-close barrier (gated by the last V copies).
        ps_pj = ctx.enter_context(tc.tile_pool(name="pspj", bufs=2, space="PSUM"))
        vts = {}
        with tc.tile_pool(name="psv", bufs=1, space="PSUM") as ps_v:
            vps = [
                ps_v.tile([128, DHG], F32, tag=f"v{slot}", name=f"vps{slot}")
                for slot in range(6)
            ]
            # HAM warm-up on a memset scratch tile while the first input
            # DMAs land: a few cold matmuls of dead work lift the PE clock
            # gate right as the real stream begins.
            for i in range(9):
                nc.tensor.matmul(
                    vps[i % 4][0:1, :], ones[:], scratch[:], start=True, stop=True
                )
            # 16 seq-chunk accumulators over 6 PSUM banks -> 3 passes.
            passes = [list(range(0, 6)), list(range(6, 12)), list(range(12, 16))]
            for pi, stgs in enumerate(passes):
                if pi:
                    vps = [
                        ps_v.tile([128, DHG], F32, tag=f"v{slot}",
                                  name=f"vps{pi}_{slot}")
                        for slot in range(len(stgs))
                    ]
                for d in range(ND):
                    for j, st in enumerate(stgs):
                        nc.tensor.matmul(
                            vps[j][:],
                            xts[d][st // 4][:, (st % 4) * 128:(st % 4 + 1) * 128],
                            ws["wv"][d][:],
                            start=(d == 0),
                            stop=(d == ND - 1),
                        )
                # copies alternate ACT/DVE so the wave drains two at a time.
                for j, st in enumerate(stgs):
                    vt = p_v.tile([128, DHG], F16, tag="v", name=f"vt{st}")
                    if j % 2 == 0:
                        nc.scalar.copy(vt[:], vps[j][:])
                    else:
                        nc.vector.tensor_copy(vt[:], vps[j][:])
                    vts[st] = vt
        vts = [vts[st] for st in range(16)]

        ps_mm = ctx.enter_context(tc.tile_pool(name="psmm", bufs=2, space="PSUM"))
        ps_pv = ctx.enter_context(tc.tile_pool(name="pspv", bufs=1, space="PSUM"))

        def proj_steps(h, defer_kt_tail=False):
            """Yield once per PE-chunk of head h's q/k projections.

            With defer_kt_tail, yields a ("ready", qt, kt) marker after qt
            and the first kt block — the remaining kt blocks are pulled by
            head h's OWN attention loop (whose inner loop is ACT-paced with
            PE slack), instead of head h-1's PE-bound window.
            """
            qt = p_qt.tile([128, S], F16, tag="qt", name=f"qt{h}")
            kt = p_qt.tile([128, S], F16, tag="kt", name=f"kt{h}")

            def block(dst, wname, scale, sb, early_cast=False):
                # early_cast: no yield between the block's last matmul and
                # its PSUM->SBUF cast, so a consumer emitted right after the
                # next pull already sees the cast in program order.
                ps = ps_pj.tile([128, 512], F32, tag="proj", name=f"pj{h}")
                for d in range(ND):
                    nc.tensor.matmul(
                        ps[:],
                        ws[wname][d][:, h * 128:(h + 1) * 128],
                        xts[d][sb][:],
                        start=(d == 0),
                        stop=(d == ND - 1),
                    )
                    if d % 2 == 1 and not (early_cast and d == ND - 1):
                        yield None
                dsl = dst[:, sb * 512:(sb + 1) * 512]
                if scale is not None:
                    nc.vector.tensor_scalar_mul(dsl, ps[:], scale)
                else:
                    nc.vector.tensor_copy(dsl, ps[:])

            for sb in range(S // 512):
                yield from block(qt, "wq", SCALE, sb)
            yield from block(kt, "wk", None, 0)
            if defer_kt_tail:
                yield ("ready", qt, kt)
            for sb in range(1, S // 512):
                yield from block(kt, "wk", None, sb, early_cast=defer_kt_tail)
            while True:
                yield (qt, kt)

        class Drip:
            """Pulls a proj generator until its ready/done marker, then
            stops (so a deferred tail isn't consumed by the wrong head)."""

            def __init__(self, gen):
                self.gen = gen
                self.res = None

            def step(self):
                if self.res is None:
                    r = next(self.gen)
                    if isinstance(r, tuple):
                        self.res = r

            def drain(self):
                while self.res is None:
                    self.step()
                return self.res

        def attention_head(h, qt, kt, next_proj, tail_proj=None):
            """Phase B for head h; drip-feeds next head's projection matmuls
            into the ACT-paced kt loop. tail_proj (last head only) supplies
            this head's own deferred kt blocks: block sb_j must land before
            QK(4j), so it is pulled at 2 steps/iteration over k=2..7."""
            for qb in range(S // 1024):
                is_last = h == HPC - 1 and qb == 1
                pv = ps_pv.tile([128, 1024], F32, tag="pv")
                q0 = qb * 1024
                ets = {}
                acc = [None]

                def qk_step(k, split_exp=False):
                    st_ps = ps_mm.tile([128, 1024], F32, tag="sT")
                    for hf in range(2):
                        nc.tensor.matmul(
                            st_ps[:, hf * 512:(hf + 1) * 512],
                            kt[:, k * 128:(k + 1) * 128],
                            qt[:, q0 + hf * 512:q0 + (hf + 1) * 512],
                            start=True,
                            stop=True,
                        )
                    et = p_exp.tile([128, 1024], F16, tag="exp")
                    if split_exp:
                        # kernel-final EXP in halves: the PV stop-matmul and
                        # the tail cast chain start half an EXP earlier; the
                        # extra ACT instruction overhead lands after the ACT
                        # stream is otherwise done.
                        for hf in range(2):
                            sl = slice(hf * 512, (hf + 1) * 512)
                            nc.scalar.activation(
                                et[:, sl], st_ps[:, sl],
                                mybir.ActivationFunctionType.Exp,
                            )
                            nc.vector.tensor_add(
                                acc[0][:, sl], acc[0][:, sl], et[:, sl]
                            )
                        ets[k] = et
                        return
                    nc.scalar.activation(
                        et[:], st_ps[:], mybir.ActivationFunctionType.Exp
                    )
                    ets[k] = et
                    # Fold the exp-sum accumulator on DVE right away,
                    # decoupled from the PV matmuls: the per-block sum
                    # matmuls then depend only on (ACT, DVE) progress and
                    # never stall the PE FIFO behind a lagging fold.
                    if k == 1:
                        acc[0] = p_exp.tile(
                            [128, 1024], F16, tag="acc", name=f"acc{h}q{qb}",
                            bufs=2 * HPC,
                        )
                        nc.vector.tensor_add(acc[0][:], ets[0][:], et[:])
                    elif k > 1:
                        nc.vector.tensor_add(acc[0][:], acc[0][:], et[:])

                def pv_step(k):
                    et = ets.pop(k)
                    for hf in range(2):
                        sl = slice(hf * 512, (hf + 1) * 512)
                        nc.tensor.matmul(
                            pv[:, sl],
                            vts[k][:, h * 128:(h + 1) * 128],
                            et[:, sl],
                            start=(k == 0),
                            stop=(k == KT - 1),
                        )

                def sum_block(b, a, split=False):
                    # cross-partition exp-sum of block b's accumulator to a
                    # [1,1024] f32 row in DRAM; the host divides. split:
                    # stage the halves on ACT+DVE in parallel (tail path).
                    sms = [
                        ps_pj.tile([1, 512], F32, tag="proj", name=f"sm{b}{hf}")
                        for hf in range(2)
                    ]
                    for hf in range(2):
                        sl = slice(hf * 512, (hf + 1) * 512)
                        nc.tensor.matmul(
                            sms[hf][:], ones[:], a[:, sl],
                            start=True, stop=True,
                        )
                    sm_sb = p_rc.tile([1, 1024], F32, tag="smr", name=f"smsb{b}")
                    for hf in range(2):
                        sl = slice(hf * 512, (hf + 1) * 512)
                        if split and hf == 0:
                            nc.scalar.copy(sm_sb[:, sl], sms[hf][:])
                        else:
                            nc.vector.tensor_copy(sm_sb[:, sl], sms[hf][:])
                    nc.sync.dma_start(sums_ap[b:b + 1, :], sm_sb[:])

                qk_step(0)
                qk_step(1)
                for k in range(2, KT):
                    pv_step(k - 2)
                    qk_step(k, split_exp=(is_last and k == KT - 1))
                    if next_proj is not None:
                        next_proj.step()
                    if tail_proj is not None and qb == 0 and k <= 7:
                        next(tail_proj)
                        next(tail_proj)
                    if (h == HPC - 1 and deferred_sums and
                            (k >= 10 if qb == 0 else 4 <= k <= 10)):
                        # the last head's loops are ACT-paced with PE slack
                        # (qb0 after the proj-tail drain, qb1 throughout):
                        # earlier blocks' sum matmuls fill the idle slots.
                        sum_block(*deferred_sums.pop(0))
                        if qb == 1 and deferred_sums:
                            sum_block(*deferred_sums.pop(0))
                pv_step(KT - 2)
                pv_step(KT - 1)

                # Unnormalized output: PSUM -> f16 (frees the PV bank for
                # the next block), plus the cross-partition exp-sums to a
                # [1,1024] f32 row — the host divides during unsharding.
                ob = p_out.tile([128, 1024], F16, tag="o")
                if is_last:
                    # ACT is idle after its final EXP; split the output
                    # cast across ACT/DVE and the store across both
                    # HWDGE queues so the tail drains in parallel.
                    nc.scalar.copy(ob[:, 0:512], pv[:, 0:512])
                    nc.vector.tensor_copy(ob[:, 512:1024], pv[:, 512:1024])
                    sum_block(h * 2 + qb, acc[0][:], split=True)
                    nc.sync.dma_start(
                        out_ap[h * 128:(h + 1) * 128, q0:q0 + 512],
                        ob[:, 0:512],
                    )
                    nc.scalar.dma_start(
                        out_ap[h * 128:(h + 1) * 128, q0 + 512:q0 + 1024],
                        ob[:, 512:1024],
                    )
                else:
                    nc.vector.tensor_copy(ob[:], pv[:])
                    deferred_sums.append((h * 2 + qb, acc[0][:]))
                    nc.sync.dma_start(
                        out_ap[h * 128:(h + 1) * 128, q0:q0 + 1024], ob[:]
                    )

        deferred_sums = []

        # head 0's projections run serially (nothing to hide them under);
        # heads 1..3 project inside the previous head's attention loop,
        # except the last head's kt tail which projects inside its own.
        gen = Drip(proj_steps(0))
        qt, kt = gen.drain()
        tail_gen = None
        for h in range(HPC):
            last = h == HPC - 1
            nxt = (
                Drip(proj_steps(h + 1, defer_kt_tail=(h + 1 == HPC - 1)))
                if not last
                else None
            )
            attention_head(h, qt, kt, nxt, tail_gen if last else None)
            if nxt is not None:
                r = nxt.drain()
                if len(r) == 3 and isinstance(r[0], str):
                    qt, kt = r[1], r[2]
                    tail_gen = nxt.gen
                else:
                    qt, kt = r


def _build():
    nc = bacc.Bacc(
        "TRN2",
        target_bir_lowering=False,
        debug=False,
        enable_asserts=False,
        num_devices=N_CORES,
    )
    # Panel layouts (one 128-partition band, contraction chunks side by
    # side) so every DMA moves >=4KB per partition line — descriptor count,
    # not bytes, is what gates the input load.
    xt_ap = nc.dram_tensor("xt", [128, (D // 128) * S], F16, kind="ExternalInput").ap()
    wq_ap = nc.dram_tensor("wq", [128, (D // 128) * DHG], F16, kind="ExternalInput").ap()
    wk_ap = nc.dram_tensor("wk", [128, (D // 128) * DHG], F16, kind="ExternalInput").ap()
    wv_ap = nc.dram_tensor("wv", [128, (D // 128) * DHG], F16, kind="ExternalInput").ap()
    out_ap = nc.dram_tensor("out", [DHG, S], F16, kind="ExternalOutput").ap()
    sums_ap = nc.dram_tensor(
        "sums", [2 * HPC, 1024], F32, kind="ExternalOutput"
    ).ap()
    with tile.TileContext(nc) as tc:
        _emit(tc, nc, xt_ap, wq_ap, wk_ap, wv_ap, out_ap, sums_ap)
    nc.compile()
    return nc


def _shard_inputs(inputs):
    x = np.ascontiguousarray(np.asarray(inputs["input_embeddings"], dtype=np.float32))
    wq = np.asarray(inputs["w_query"], dtype=np.float32)
    wk = np.asarray(inputs["w_key"], dtype=np.float32)
    wv = np.asarray(inputs["w_value"], dtype=np.float32)

    def panel(a):
        # [D, W] -> [128, (D//128)*W]: contraction chunks side by side so a
        # per-chunk DMA reads one contiguous W-wide line per partition.
        Dd, W = a.shape
        return np.ascontiguousarray(
            a.reshape(Dd // 128, 128, W).transpose(1, 0, 2).reshape(128, -1)
        )

    def gather(w, g):
        # head h occupies the strided cols d = hd*8 + h; regroup head-major
        w4 = w.reshape(D, DH, H)[:, :, g * HPC:(g + 1) * HPC]   # (D, hd, hl)
        return panel(
            w4.transpose(0, 2, 1).reshape(D, DHG).astype(np.float16)
        )

    in_maps = []
    xtp = [panel(x[b].T.astype(np.float16)) for b in range(B)]
    for c in range(N_CORES):
        b, g = divmod(c, 2)
        in_maps.append(
            {
                "xt": xtp[b],
                "wq": gather(wq, g),
                "wk": gather(wk, g),
                "wv": gather(wv, g),
            }
        )
    return in_maps


def kernel(**inputs):
    nc = _CACHE.get("nc")
    if nc is None:
        nc = _CACHE["nc"] = _build()
    in_maps = _shard_inputs(inputs)
    res = run_bass_kernel_spmd(
        nc, in_maps, core_ids=list(range(N_CORES)), trace=TRACE
    )
    _CACHE["last_result"] = res
    out = np.empty((B, S, DH, H), dtype=np.float32)
    for c in range(N_CORES):
        b, g = divmod(c, 2)
        o = res.results[c]["out"].astype(np.float32).reshape(HPC, DH, S)
        # softmax normalization: divide by the exp-sums exported per
        # (head, query-block); sums row b covers head b//2, cols b%2.
        s = res.results[c]["sums"].astype(np.float32).reshape(HPC, 1, S)
        o /= s
        out[b, :, :, g * HPC:(g + 1) * HPC] = o.transpose(2, 1, 0)
    return out.reshape(B, S, D)
